# revision 1
# baseline (speedup 1.0000x reference)
"""Multi-head attention (B=2, S=2048, D=1024, H=16) on 8 Trainium2 cores.

Sharding: 2-way data parallel on batch x 4-way tensor parallel on heads.
Core c handles batch b = c // 4 and head group g = c % 4 (4 heads, 256 dims).

Per-core pipeline (all "feature-major" / transposed layouts so that every
matmul streams a long free dim and biases land on partitions):
  XT [1024, 2048]  (host-transposed input slice, bf16)
  QT = (Wq.T @ X.T + bq) / 8   [256, 2048]
  KT =  Wk.T @ X.T + bk        [256, 2048]
  V  =  X @ Wv + bv            [2048, 256]   (bias via ones-row matmul)
  per head h:
    scoresT[t, s] = KT_h[:, t]_tile.T @ QT_h      (PE, K=64)
    expT = exp(scoresT)                           (ACT, PSUM -> SBUF bf16)
    hoT'[65, s]  = [V_h | 1].T @ expT             (PE, accumulate over t)
      rows 0:64 = unnormalized head out (d, s), row 64 = sumexp[s]
    hoT = hoT' / sumexp                           (GpSimd bcast + DVE)
  outT[e, s] = Wout_g.T-ish: lhsT=Wout_g tiles, rhs=hoT  (partial over group)
Host sums the 4 per-group partials per batch and transposes back.
"""

import numpy as np
import ml_dtypes

BF16 = ml_dtypes.bfloat16

S = 2048  # sequence length
C = 1024  # d_model
NH = 16  # total heads
DK = 64  # head dim
N_CORES = 8
HPC = 4  # heads per core
DH = HPC * DK  # 256 per-core head dims
P = 128
VW = 72  # per-head stride in the V' buffer: 64 V cols + 1 ones col + 7 pad

_CACHE = {}


def _build_program():
    import concourse.bacc as bacc
    import concourse.mybir as mybir
    import concourse.tile as tile
    from contextlib import ExitStack

    dt = mybir.dt
    AF = mybir.ActivationFunctionType

    nc = bacc.Bacc("TRN2", target_bir_lowering=False, debug=False,
                   num_devices=N_CORES)

    xt = nc.dram_tensor("xt", [C, S], dt.bfloat16, kind="ExternalInput")
    wq = nc.dram_tensor("wq", [C, DH], dt.bfloat16, kind="ExternalInput")
    wk = nc.dram_tensor("wk", [C, DH], dt.bfloat16, kind="ExternalInput")
    wv = nc.dram_tensor("wv", [C, DH], dt.bfloat16, kind="ExternalInput")
    wo = nc.dram_tensor("wo", [DH, C], dt.bfloat16, kind="ExternalInput")
    # cols: [bq_tile0/8, bq_tile1/8, bk_tile0, bk_tile1]
    bqk = nc.dram_tensor("bqk", [P, 4], dt.float32, kind="ExternalInput")
    bv = nc.dram_tensor("bv", [1, DH], dt.bfloat16, kind="ExternalInput")
    outT = nc.dram_tensor("outT", [C, S], dt.bfloat16, kind="ExternalOutput")

    xt_r = xt.rearrange("(n p) s -> n p s", p=P)  # [8, 128, 2048]
    wq_r = wq.rearrange("(n p) d -> n p d", p=P)  # [8, 128, 256]
    wk_r = wk.rearrange("(n p) d -> n p d", p=P)
    wv_r = wv.rearrange("(n p) d -> n p d", p=P)
    wo_r = wo.rearrange("(n p) e -> n p e", p=P)  # [2, 128, 1024]
    outT_r = outT.rearrange("(n p) s -> n p s", p=P)  # [8, 128, 2048]

    with ExitStack() as ctx:
        tc = ctx.enter_context(tile.TileContext(nc))
        sb = ctx.enter_context(tc.tile_pool(name="sb", bufs=1))
        xpool = ctx.enter_context(tc.tile_pool(name="xpool", bufs=1))
        pool2 = ctx.enter_context(tc.tile_pool(name="pool2", bufs=1))
        spool = ctx.enter_context(tc.tile_pool(name="spool", bufs=2, space="PSUM"))
        vpool = ctx.enter_context(tc.tile_pool(name="vpool", bufs=1, space="PSUM"))

        # ---- persistent SBUF ----
        qt_sb = [sb.tile([P, S], dt.bfloat16, name=f"qt{i}", tag=f"qt{i}") for i in range(2)]
        kt_sb = [sb.tile([P, S], dt.bfloat16, name=f"kt{i}", tag=f"kt{i}") for i in range(2)]
        v_sb = [sb.tile([P, HPC * VW], dt.bfloat16, name=f"v{i}", tag=f"v{i}") for i in range(16)]
        exp_sb = [sb.tile([P, S], dt.bfloat16, name=f"e{i}", tag=f"e{i}") for i in range(16)]
        hot_sb = [sb.tile([P, S], dt.bfloat16, name=f"ho{i}", tag=f"ho{i}") for i in range(2)]
        wo_sb = [sb.tile([P, C], dt.bfloat16, name=f"wo{i}", tag=f"wo{i}") for i in range(2)]
        bqk_sb = sb.tile([P, 4], dt.float32, name="bqk", tag="bqk")
        bv_sb = sb.tile([1, DH], dt.bfloat16, name="bv", tag="bv")
        ones_sb = sb.tile([1, P], dt.bfloat16, name="ones", tag="ones")

        # ---- phase-1-only SBUF ----
        xt_sb = [xpool.tile([P, S], dt.bfloat16, name=f"x{i}", tag=f"x{i}") for i in range(8)]
        wq_sb = [xpool.tile([P, DH], dt.bfloat16, name=f"wq{i}", tag=f"wq{i}") for i in range(8)]
        wk_sb = [xpool.tile([P, DH], dt.bfloat16, name=f"wk{i}", tag=f"wk{i}") for i in range(8)]
        wv_sb = [xpool.tile([P, DH], dt.bfloat16, name=f"wv{i}", tag=f"wv{i}") for i in range(8)]

        # ---- loads: xt on the two HWDGE queues (sync + scalar), weights on
        # the gpsimd software DGE so no compute engine burns time on DMA
        # issue mid-phase. xt comes in s-column halves so the first QKT
        # chunk can start after half the transfer ----
        for half in range(2):
            cs = slice(half * 1024, (half + 1) * 1024)
            for i in range(8):
                eng = nc.sync if i % 2 == 0 else nc.scalar
                eng.dma_start(out=xt_sb[i][:, cs], in_=xt_r[i][:, cs])
        for i in range(8):
            nc.gpsimd.dma_start(out=wq_sb[i], in_=wq_r[i])
        nc.gpsimd.dma_start(out=bqk_sb, in_=bqk[:, :])
        nc.gpsimd.dma_start(out=bv_sb, in_=bv[:, :])
        for i in range(8):
            nc.gpsimd.dma_start(out=wk_sb[i], in_=wk_r[i])
        for i in range(8):
            nc.gpsimd.dma_start(out=wv_sb[i], in_=wv_r[i])
        for i in range(2):
            nc.sync.dma_start(out=wo_sb[i], in_=wo_r[i])
        nc.vector.memset(ones_sb, 1.0)
        # pre-set the per-head ones column in each V' tile (cols h*VW + DK)
        for t in range(16):
            col = v_sb[t].rearrange("p (h w) -> p h w", w=VW)[:, :, DK:DK + 1]
            nc.vector.memset(col, 1.0)

        def qkt_unit(d2, ch, qk):
            """One [128, 1024] chunk of QT or KT for d-tile d2. The
            1/sqrt(dk) scale is folded into Wq host-side, so the epilogue is
            a plain bias-add on DVE (keeps ACT free for exp)."""
            dst, w_sb, bias_col = (
                (qt_sb, wq_sb, 0) if qk == 0 else (kt_sb, wk_sb, 2)
            )
            ps = spool.tile([P, 1024], dt.float32, name="mm", tag="mm")
            for half in range(2):
                for c8 in range(8):
                    nc.tensor.matmul(
                        ps[:, half * 512:(half + 1) * 512],
                        lhsT=w_sb[c8][:, d2 * P:(d2 + 1) * P],
                        rhs=xt_sb[c8][:, ch * 1024 + half * 512:
                                      ch * 1024 + (half + 1) * 512],
                        start=(c8 == 0), stop=(c8 == 7),
                    )
            nc.vector.tensor_scalar_add(
                dst[d2][:, ch * 1024:(ch + 1) * 1024], ps,
                bqk_sb[:, bias_col + d2:bias_col + d2 + 1],
            )

        def qkt_half(d2):
            for ch in range(2):  # ch outer chases the half-column xt DMAs
                for qk in range(2):
                    qkt_unit(d2, ch, qk)

        def vproj_t(t):
            # V tile t: [128, 256] + bias via ones-row; packed [64|1|pad] x4.
            # Ones columns were pre-set at startup; the epilogue is a single
            # strided DVE copy so PE stays the pacer.
            ps = spool.tile([P, DH], dt.float32, name="mm", tag="mm")
            for c8 in range(8):
                nc.tensor.matmul(
                    ps, lhsT=xt_sb[c8][:, t * P:(t + 1) * P],
                    rhs=wv_sb[c8], start=(c8 == 0), stop=False,
                )
            nc.tensor.matmul(ps, lhsT=ones_sb, rhs=bv_sb,
                             start=False, stop=True)
            dst = v_sb[t].rearrange("p (h w) -> p h w", w=VW)[:, :, 0:DK]
            src = ps.rearrange("p (h w) -> p h w", w=DK)
            nc.vector.tensor_copy(dst, src)

        def scores_t(h, t):
            half_idx = h // 2
            row0 = (h % 2) * DK
            kth = kt_sb[half_idx]
            qth = qt_sb[half_idx]
            for ch in range(2):
                ps = spool.tile([P, 1024], dt.float32, name="mm", tag="mm")
                for half in range(2):
                    s0 = ch * 1024 + half * 512
                    nc.tensor.matmul(
                        ps[:, half * 512:(half + 1) * 512],
                        lhsT=kth[row0:row0 + DK, t * P:(t + 1) * P],
                        rhs=qth[row0:row0 + DK, s0:s0 + 512],
                        start=True, stop=True,
                    )
                nc.scalar.activation(
                    exp_sb[t][:, ch * 1024:(ch + 1) * 1024], ps, AF.Exp
                )

        def scores(h):
            for t in range(16):
                scores_t(h, t)

        sumtmp = pool2.tile([1, S], dt.float32, name="sumtmp", tag="sumtmp")
        sr = pool2.tile([1, S], dt.float32, name="sr", tag="sr")
        rbc = pool2.tile([DK, S], dt.float32, name="rbc", tag="rbc")

        def attn_block(h):
            # attn_h @ [V | 1] interleaved per-t with scores of head h+1:
            # ACT stays saturated with exp work through the whole stream.
            # t OUTER on attnV so each exp tile is fully consumed after 4
            # back-to-back matmuls (releases the WAR for head h+1's exp
            # writes immediately -- no pipeline convoy). The 4 s-chunk
            # accumulators live in one 4-bank PSUM tile.
            half_idx = h // 2
            row0 = (h % 2) * DK
            pv = vpool.tile([DK + 1, S], dt.float32, name="av", tag="av")
            for t in range(16):
                for ch4 in range(4):
                    nc.tensor.matmul(
                        pv[:, ch4 * 512:(ch4 + 1) * 512],
                        lhsT=v_sb[t][:, h * VW:h * VW + DK + 1],
                        rhs=exp_sb[t][:, ch4 * 512:(ch4 + 1) * 512],
                        start=(t == 0), stop=(t == 15),
                    )
                if h + 1 < HPC:
                    scores_t(h + 1, t)
            # normalization, chunk-pipelined: sumexp row (PSUM partition 64)
            # -> partition 0, approx reciprocal (custom DVE op needs base
            # partition 0), broadcast across 64 partitions, then one wide
            # multiply straight out of PSUM into the bf16 hoT buffer.
            for ch4 in range(4):
                c = slice(ch4 * 512, (ch4 + 1) * 512)
                nc.vector.tensor_copy(sumtmp[:, c], pv[DK:DK + 1, c])
                nc.vector.reciprocal_approx_fast(sr[:, c], sumtmp[:, c])
                nc.gpsimd.partition_broadcast(rbc[:, c], sr[:, c])
            nc.vector.tensor_mul(
                hot_sb[half_idx][row0:row0 + DK, :], pv[:DK, :], rbc,
            )

        def outproj():
            for e in range(8):
                for ch in range(2):
                    ps = spool.tile([P, 1024], dt.float32, name="mm", tag="mm")
                    for half in range(2):
                        s0 = ch * 1024 + half * 512
                        for d2 in range(2):
                            nc.tensor.matmul(
                                ps[:, half * 512:(half + 1) * 512],
                                lhsT=wo_sb[d2][:, e * P:(e + 1) * P],
                                rhs=hot_sb[d2][:, s0:s0 + 512],
                                start=(d2 == 0), stop=(d2 == 1),
                            )
                    st = pool2.tile([P, 1024], dt.bfloat16, name="st",
                                    tag="st", bufs=3)
                    if (e + ch) % 2 == 0:
                        nc.vector.tensor_copy(st, ps)
                        nc.sync.dma_start(
                            out=outT_r[e][:, ch * 1024:(ch + 1) * 1024],
                            in_=st)
                    else:
                        nc.scalar.copy(st, ps)
                        nc.scalar.dma_start(
                            out=outT_r[e][:, ch * 1024:(ch + 1) * 1024],
                            in_=st)

        # Emission order: QKT half 0 first (paced by the xt DMA), then a
        # PE-dense stream [scores0 | V | QKT half 1] that puts exp work on
        # ACT as early as possible, then the attention blocks (scores of
        # head h+1 ride inside head h's attnV) so ACT never starves.
        qkt_half(0)
        for t in range(16):
            scores_t(0, t)
            vproj_t(t)
            if t % 4 == 0:
                u = t // 4
                qkt_unit(1, u // 2, u % 2)
        attn_block(0)
        attn_block(1)
        attn_block(2)
        attn_block(3)
        outproj()

    nc.compile()
    return nc


def _get_program():
    if "nc" not in _CACHE:
        _CACHE["nc"] = _build_program()
    return _CACHE["nc"]


def _shard_inputs(input, W_qkv, b_qkv, W_out):
    """Build the 8 per-core input maps (host-side shard + transpose + cast)."""
    in_maps = []
    xt_by_b = [
        np.ascontiguousarray(input[b].T).astype(BF16) for b in range(2)
    ]
    for core in range(N_CORES):
        b, g = divmod(core, HPC)
        cols = slice(g * DH, (g + 1) * DH)
        bq = (b_qkv[g * DH:(g + 1) * DH] / 8.0).astype(np.float32)
        bk = b_qkv[C + g * DH:C + (g + 1) * DH].astype(np.float32)
        bqk = np.stack([bq[:P], bq[P:], bk[:P], bk[P:]], axis=1)
        in_maps.append({
            "xt": xt_by_b[b],
            "wq": np.ascontiguousarray(W_qkv[:, cols] * 0.125).astype(BF16),
            "wk": np.ascontiguousarray(W_qkv[:, C:2 * C][:, cols]).astype(BF16),
            "wv": np.ascontiguousarray(W_qkv[:, 2 * C:][:, cols]).astype(BF16),
            "wo": np.ascontiguousarray(W_out[g * DH:(g + 1) * DH, :]).astype(BF16),
            "bqk": np.ascontiguousarray(bqk, dtype=np.float32),
            "bv": b_qkv[2 * C + g * DH:2 * C + (g + 1) * DH]
                  .astype(BF16).reshape(1, DH),
        })
    return in_maps


def kernel(input, W_qkv, b_qkv, W_out):
    from concourse.bass_utils import run_bass_kernel_spmd

    nc = _get_program()
    in_maps = _shard_inputs(
        np.asarray(input), np.asarray(W_qkv), np.asarray(b_qkv),
        np.asarray(W_out),
    )
    res = run_bass_kernel_spmd(nc, in_maps, core_ids=list(range(N_CORES)))
    out = np.zeros((2, S, C), dtype=np.float32)
    for core in range(N_CORES):
        b = core // HPC
        out[b] += np.asarray(res.results[core]["outT"]).astype(np.float32).T
    return out



# revision 12
# speedup vs baseline: 1.0942x; 1.0942x over previous
"""Multi-head attention (B=2, S=2048, D=1024, H=16) on 8 Trainium2 cores.

Sharding: 2-way data parallel on batch x 4-way tensor parallel on heads.
Core c handles batch b = c // 4 and head group g = c % 4 (4 heads, 256 dims).

Key ideas over the v1 kernel (which was ACT-bound in the attention phase):
  - exp() work is split between ACT and DVE: s-columns 0:1024 of every
    scores tile go through ACT's Exp; columns 1024:2048 are computed on
    DVE with the Schraudolph bit-trick (int32(A*x+B) reinterpreted as
    fp32), stored as int32 and fed to the attn@V matmul as float32r
    (1 cycle/row for N>=256, same PE cost as bf16).
  - attn@V accumulates per s-half in [65, 1024] PSUM tiles (2+2 banks,
    double-buffered) with the sumexp "ones" column FIRST, so
    reciprocal_approx_fast can read PSUM partition 0 directly.
  - V bias is added on DVE during the PSUM->SBUF copy (against a
    gpsimd-broadcast bias tile) instead of a ones-row matmul.
  - xt streams in s-column quarters across 3 HWDGE queues so the first
    QKT matmul can start at ~9us and never starves.
"""

import numpy as np
import ml_dtypes

BF16 = ml_dtypes.bfloat16

S = 2048  # sequence length
C = 1024  # d_model
NH = 16  # total heads
DK = 64  # head dim
N_CORES = 8
HPC = 4  # heads per core
DH = HPC * DK  # 256 per-core head dims
P = 128
VW = 72  # per-head stride in V': 64 V cols + 1 ones col + 7 pad

# Schraudolph exp, bf16 variant: exp(x) ~= bitcast_bf16(int16(x*EXPA + EXPB))
# (the int16 value is the bf16 bit pattern; 2^-16 folds the fp32 trick down)
EXPA = float(2**23 / np.log(2.0) / 65536.0)
EXPB = float(((127 << 23) - 90000) / 65536.0)

_CACHE = {}


def _build_program():
    import concourse.bacc as bacc
    import concourse.mybir as mybir
    import concourse.tile as tile
    from contextlib import ExitStack

    dt = mybir.dt
    AF = mybir.ActivationFunctionType
    ALU = mybir.AluOpType

    nc = bacc.Bacc("TRN2", target_bir_lowering=False, debug=False,
                   num_devices=N_CORES)

    xt = nc.dram_tensor("xt", [C, S], dt.bfloat16, kind="ExternalInput")
    wq = nc.dram_tensor("wq", [C, DH], dt.bfloat16, kind="ExternalInput")
    wk = nc.dram_tensor("wk", [C, DH], dt.bfloat16, kind="ExternalInput")
    wv = nc.dram_tensor("wv", [C, DH], dt.bfloat16, kind="ExternalInput")
    wo = nc.dram_tensor("wo", [DH, C], dt.bfloat16, kind="ExternalInput")
    # cols: [bq_tile0/8, bq_tile1/8, bk_tile0, bk_tile1]
    bqk = nc.dram_tensor("bqk", [P, 4], dt.float32, kind="ExternalInput")
    bv = nc.dram_tensor("bv", [1, DH], dt.bfloat16, kind="ExternalInput")
    outT = nc.dram_tensor("outT", [C, S], dt.bfloat16, kind="ExternalOutput")

    xt_d = xt.rearrange("(n p) s -> p n s", p=P)  # [128, 8, 2048]
    wq_d = wq.rearrange("(n p) d -> p n d", p=P)  # [128, 8, 256]
    wk_d = wk.rearrange("(n p) d -> p n d", p=P)
    wv_d = wv.rearrange("(n p) d -> p n d", p=P)
    wo_d = wo.rearrange("(n p) e -> p n e", p=P)  # [128, 2, 1024]
    outT_r = outT.rearrange("(n p) s -> n p s", p=P)  # [8, 128, 2048]

    with ExitStack() as ctx:
        tc = ctx.enter_context(tile.TileContext(nc))
        sb = ctx.enter_context(tc.tile_pool(name="sb", bufs=1))
        xpool = ctx.enter_context(tc.tile_pool(name="xpool", bufs=1))
        pool2 = ctx.enter_context(tc.tile_pool(name="pool2", bufs=1))
        spool = ctx.enter_context(tc.tile_pool(name="spool", bufs=2, space="PSUM"))
        vpool = ctx.enter_context(tc.tile_pool(name="vpool", bufs=2, space="PSUM"))

        # ---- persistent SBUF ----
        qt_sb = [sb.tile([P, S], dt.bfloat16, name=f"qt{i}", tag=f"qt{i}") for i in range(2)]
        kt_sb = [sb.tile([P, S], dt.bfloat16, name=f"kt{i}", tag=f"kt{i}") for i in range(2)]
        v_sb = [sb.tile([P, HPC * VW], dt.bfloat16, name=f"v{i}", tag=f"v{i}") for i in range(16)]
        expa_sb = [sb.tile([P, S // 2], dt.bfloat16, name=f"ea{i}", tag=f"ea{i}") for i in range(16)]
        expb_sb = [sb.tile([P, S // 2], dt.int16, name=f"eb{i}", tag=f"eb{i}") for i in range(16)]
        hot_sb = [sb.tile([P, S], dt.bfloat16, name=f"ho{i}", tag=f"ho{i}") for i in range(2)]
        wo_sb = sb.tile([P, 2 * C], dt.bfloat16, name="wo", tag="wo")
        bqk_sb = sb.tile([P, 4], dt.float32, name="bqk", tag="bqk")
        bv_sb = sb.tile([1, DH], dt.bfloat16, name="bv", tag="bv")
        bvb_sb = sb.tile([P, DH], dt.bfloat16, name="bvb", tag="bvb")

        # ---- phase-1-only SBUF ----
        xt_sb = xpool.tile([P, 8 * S], dt.bfloat16, name="x", tag="x")
        wq_sb = xpool.tile([P, 8 * DH], dt.bfloat16, name="wq", tag="wq")
        wk_sb = xpool.tile([P, 8 * DH], dt.bfloat16, name="wk", tag="wk")
        wv_sb = xpool.tile([P, 8 * DH], dt.bfloat16, name="wv", tag="wv")

        xt_v = xt_sb.rearrange("p (n s) -> p n s", s=S)  # [128, 8, 2048]
        wq_v = wq_sb.rearrange("p (n d) -> p n d", d=DH)
        wk_v = wk_sb.rearrange("p (n d) -> p n d", d=DH)
        wv_v = wv_sb.rearrange("p (n d) -> p n d", d=DH)
        wo_v = wo_sb.rearrange("p (n e) -> p n e", e=C)

        # ---- DMA kicks: xt in s-column quarters so QKT can start early.
        # sync: bqk + wq + x tiles 0-2; scalar: x tiles 3-7 (balanced vs
        # sync's weight prefix); gpsimd swdge: wk/bv/wv/wo (needed later).
        nc.gpsimd.dma_start(out=wk_sb.rearrange("p (n d) -> p n d", d=DH),
                            in_=wk_d)
        nc.gpsimd.dma_start(out=bv_sb, in_=bv[:, :])
        nc.gpsimd.dma_start(out=wv_sb.rearrange("p (n d) -> p n d", d=DH),
                            in_=wv_d)
        nc.gpsimd.dma_start(out=wo_v, in_=wo_d)
        nc.sync.dma_start(out=bqk_sb, in_=bqk[:, :])
        nc.sync.dma_start(out=wq_sb.rearrange("p (n d) -> p n d", d=DH),
                          in_=wq_d)
        for q in range(4):
            qs = slice(q * 512, (q + 1) * 512)
            nc.sync.dma_start(out=xt_v[:, 0:3, qs], in_=xt_d[:, 0:3, qs])
            nc.scalar.dma_start(out=xt_v[:, 3:8, qs], in_=xt_d[:, 3:8, qs])

        # pre-set the per-head ones column in each V' tile (cols h*VW + DK)
        for t in range(16):
            col = v_sb[t].rearrange("p (h w) -> p h w", w=VW)[:, :, DK:DK + 1]
            nc.vector.memset(col, 1.0)
        # broadcast V bias across partitions for the vproj epilogue
        nc.gpsimd.partition_broadcast(bvb_sb, bv_sb)

        def qk_unit(d2, ch, qk):
            """One [128, 1024] chunk of QT or KT for d-tile d2, s-chunk ch.
            1/sqrt(dk) is folded into Wq host-side; epilogue is a DVE
            bias-add (keeps ACT free for exp)."""
            dst, w_v, bias_col = (
                (qt_sb, wq_v, 0) if qk == 0 else (kt_sb, wk_v, 2)
            )
            ps = spool.tile([P, 1024], dt.float32, name="mm", tag="mm")
            for half in range(2):
                for c8 in range(8):
                    nc.tensor.matmul(
                        ps[:, half * 512:(half + 1) * 512],
                        lhsT=w_v[:, c8, d2 * P:(d2 + 1) * P],
                        rhs=xt_v[:, c8, ch * 1024 + half * 512:
                                 ch * 1024 + (half + 1) * 512],
                        start=(c8 == 0), stop=(c8 == 7),
                    )
            nc.vector.tensor_scalar_add(
                dst[d2][:, ch * 1024:(ch + 1) * 1024], ps,
                bqk_sb[:, bias_col + d2:bias_col + d2 + 1],
            )

        def vproj_t(t):
            # V tile t: [128, 256]; bias added on DVE during the strided
            # PSUM->SBUF copy. Packed [1|64|pad] x4 heads (ones col first).
            ps = spool.tile([P, DH], dt.float32, name="mm", tag="mm")
            for c8 in range(8):
                nc.tensor.matmul(
                    ps, lhsT=xt_v[:, c8, t * P:(t + 1) * P],
                    rhs=wv_v[:, c8, :], start=(c8 == 0), stop=(c8 == 7),
                )
            dst = v_sb[t].rearrange("p (h w) -> p h w", w=VW)[:, :, 0:DK]
            src = ps.rearrange("p (h w) -> p h w", w=DK)
            bsrc = bvb_sb.rearrange("p (h w) -> p h w", w=DK)
            nc.vector.tensor_add(dst, src, bsrc)

        def scores_t(h, t):
            """Scores tile (h, t): s 0:1024 -> ACT exp (bf16); s 1024:2048
            -> DVE Schraudolph exp (int32 bits of fp32)."""
            half_idx = h // 2
            row0 = (h % 2) * DK
            kth = kt_sb[half_idx]
            qth = qt_sb[half_idx]
            for ch in range(2):
                ps = spool.tile([P, 1024], dt.float32, name="mm", tag="mm")
                for half in range(2):
                    s0 = ch * 1024 + half * 512
                    nc.tensor.matmul(
                        ps[:, half * 512:(half + 1) * 512],
                        lhsT=kth[row0:row0 + DK, t * P:(t + 1) * P],
                        rhs=qth[row0:row0 + DK, s0:s0 + 512],
                        start=True, stop=True,
                    )
                if ch == 0:
                    nc.scalar.activation(expa_sb[t], ps, AF.Exp)
                else:
                    nc.vector.tensor_scalar(
                        expb_sb[t], ps, EXPA, EXPB, ALU.mult, ALU.add,
                    )

        sum_sb = [pool2.tile([1, 1024], dt.float32, name=f"sm{i}", tag=f"sm{i}")
                  for i in range(2)]
        sr_sb = [pool2.tile([1, 1024], dt.float32, name=f"sr{i}", tag=f"sr{i}")
                 for i in range(2)]
        rbc_sb = [pool2.tile([DK, 1024], dt.float32, name=f"rbc{i}", tag=f"rbc{i}")
                  for i in range(2)]

        def attnv_half(h, sh, pv):
            """attn_h @ [1|V] for s-half sh, accumulated over the 16 t tiles
            in a [65, 1024] PSUM tile. Row 0 = sumexp. For sh=1, scores of
            head h+1 ride inside the t loop to keep ACT/DVE saturated."""
            fdt = None
            for t in range(16):
                for c2 in range(2):
                    if sh == 0:
                        rhs = expa_sb[t][:, c2 * 512:(c2 + 1) * 512]
                    else:
                        rhs = expb_sb[t][:, c2 * 512:(c2 + 1) * 512].bitcast(
                            dt.bfloat16)
                    nc.tensor.matmul(
                        pv[:, c2 * 512:(c2 + 1) * 512],
                        lhsT=v_sb[t][:, h * VW:h * VW + DK + 1],
                        rhs=rhs,
                        start=(t == 0), stop=(t == 15),
                    )
                if sh == 1 and h + 1 < HPC:
                    scores_t(h + 1, t)

        def norm_half(h, sh, pv):
            """hot[rows, s-half] = pv[0:64] / pv[64]. The sumexp row moves
            to partition 0 on ACT (it has slack; recip_approx_fast needs
            base partition 0), then gpsimd broadcast + one DVE multiply."""
            half_idx = h // 2
            row0 = (h % 2) * DK
            sm, sr, rbc = sum_sb[sh], sr_sb[sh], rbc_sb[sh]
            nc.scalar.copy(sm, pv[DK:DK + 1, :])
            nc.vector.reciprocal_approx_fast(sr, sm)
            nc.gpsimd.partition_broadcast(rbc, sr)
            nc.vector.tensor_mul(
                hot_sb[half_idx][row0:row0 + DK,
                                 sh * 1024:(sh + 1) * 1024],
                pv[0:DK, :], rbc,
            )

        def outproj():
            for ch in range(2):
                for e in range(8):
                    ps = spool.tile([P, 1024], dt.float32, name="mm", tag="mm")
                    for half in range(2):
                        s0 = ch * 1024 + half * 512
                        for d2 in range(2):
                            nc.tensor.matmul(
                                ps[:, half * 512:(half + 1) * 512],
                                lhsT=wo_v[:, d2, e * P:(e + 1) * P],
                                rhs=hot_sb[d2][:, s0:s0 + 512],
                                start=(d2 == 0), stop=(d2 == 1),
                            )
                    st = pool2.tile([P, 1024], dt.bfloat16, name="st",
                                    tag="st", bufs=3)
                    if (e + ch) % 2 == 0:
                        nc.vector.tensor_copy(st, ps)
                        nc.sync.dma_start(
                            out=outT_r[e][:, ch * 1024:(ch + 1) * 1024],
                            in_=st)
                    else:
                        nc.scalar.copy(st, ps)
                        nc.scalar.dma_start(
                            out=outT_r[e][:, ch * 1024:(ch + 1) * 1024],
                            in_=st)

        # ---- Phase 1: projections + scores(0), paced by the xt stream.
        # Q units first: wq rides the sync queue ahead of xt while wk
        # comes over the slower swdge path. ----
        qk_unit(0, 0, 0)
        qk_unit(1, 0, 0)
        qk_unit(0, 0, 1)
        qk_unit(1, 0, 1)
        for t in range(8):
            vproj_t(t)
        qk_unit(0, 1, 0)
        qk_unit(0, 1, 1)
        for t in range(8):
            scores_t(0, t)
            vproj_t(t + 8)
        scores_t(0, 8)
        scores_t(0, 9)
        qk_unit(1, 1, 0)
        scores_t(0, 10)
        scores_t(0, 11)
        scores_t(0, 12)
        qk_unit(1, 1, 1)
        scores_t(0, 13)
        scores_t(0, 14)
        scores_t(0, 15)

        # ---- Phase 2: attention blocks ----
        for h in range(HPC):
            pv0 = vpool.tile([DK + 1, 1024], dt.float32, name="av", tag="av")
            attnv_half(h, 0, pv0)
            norm_half(h, 0, pv0)
            pv1 = vpool.tile([DK + 1, 1024], dt.float32, name="av", tag="av")
            attnv_half(h, 1, pv1)
            norm_half(h, 1, pv1)

        # ---- Phase 3 ----
        outproj()

    nc.compile()
    return nc


def _get_program():
    if "nc" not in _CACHE:
        _CACHE["nc"] = _build_program()
    return _CACHE["nc"]


def _shard_inputs(input, W_qkv, b_qkv, W_out):
    """Build the 8 per-core input maps (host-side shard + transpose + cast)."""
    in_maps = []
    xt_by_b = [
        np.ascontiguousarray(input[b].T).astype(BF16) for b in range(2)
    ]
    for core in range(N_CORES):
        b, g = divmod(core, HPC)
        cols = slice(g * DH, (g + 1) * DH)
        bq = (b_qkv[g * DH:(g + 1) * DH] / 8.0).astype(np.float32)
        bk = b_qkv[C + g * DH:C + (g + 1) * DH].astype(np.float32)
        bqk = np.stack([bq[:P], bq[P:], bk[:P], bk[P:]], axis=1)
        in_maps.append({
            "xt": xt_by_b[b],
            "wq": np.ascontiguousarray(W_qkv[:, cols] * 0.125).astype(BF16),
            "wk": np.ascontiguousarray(W_qkv[:, C:2 * C][:, cols]).astype(BF16),
            "wv": np.ascontiguousarray(W_qkv[:, 2 * C:][:, cols]).astype(BF16),
            "wo": np.ascontiguousarray(W_out[g * DH:(g + 1) * DH, :]).astype(BF16),
            "bqk": np.ascontiguousarray(bqk, dtype=np.float32),
            "bv": b_qkv[2 * C + g * DH:2 * C + (g + 1) * DH]
                  .astype(BF16).reshape(1, DH),
        })
    return in_maps


def kernel(input, W_qkv, b_qkv, W_out):
    from concourse.bass_utils import run_bass_kernel_spmd

    nc = _get_program()
    in_maps = _shard_inputs(
        np.asarray(input), np.asarray(W_qkv), np.asarray(b_qkv),
        np.asarray(W_out),
    )
    res = run_bass_kernel_spmd(nc, in_maps, core_ids=list(range(N_CORES)))
    out = np.zeros((2, S, C), dtype=np.float32)
    for core in range(N_CORES):
        b = core // HPC
        out[b] += np.asarray(res.results[core]["outT"]).astype(np.float32).T
    return out


if __name__ == "__main__":
    import jax
    from reference import setup_inputs, reference

    inputs = {k: np.asarray(v) for k, v in setup_inputs().items()}
    expected = np.asarray(reference(**inputs))
    actual = kernel(**inputs)
    rel = np.linalg.norm((actual - expected).ravel()) / np.linalg.norm(
        expected.ravel())
    print("Relative error:", rel)
